# revision 1
# baseline (speedup 1.0000x reference)
"""CrossScaleAttention Trainium2 kernel.

Full inputs -> full output; shards batch (8 samples) across 8 NeuronCores,
one sample per core (pure data parallel, replicated weights).

Per-core algorithm (restructured from the reference; validated in numpy):
  - bilinear 1/3 downsample with align_corners=False == exact subsample at
    (3i+1, 3j+1).
  - score^T[p, l] = sum_{dy,dx} match_pad_win(dy,dx)^T @ ref_win(dy,dx),
    column-scaled by 10/max(||patch_l||, 1e-4); softmax over l along the free
    dim; transpose to attn[l, p] via PE transposes.
  - dynamic transposed conv decomposes into 9 residue grids (rh, rw); each is
    an accumulation over 18 units (m, n, l-chunk) of matmuls
    S_r[c, ji] += G_{r,m,n}[l, c].T @ attn_shift[l, ji], where
    G_{r,m,n}[l, c] = prelu(gather(x_pad) @ wa^T) is produced directly by the
    TensorEngine from a strided gather of padded x (fusing conv_assembly).
    attn shifts are flat offsets into the 50-wide zero-padded attn plane, so
    the matmul rhs stays a single contiguous free-dim run (N = nj*50).
  - residue planes are interleaved into full output rows in SBUF, DMA'd out.
"""

import sys

for _p in ("/opt/trn_rl_repo", "/root/.axon_site/_ro/trn_rl_repo"):
    if _p not in sys.path:
        sys.path.append(_p)

import numpy as np

import concourse.bass as bass
import concourse.tile as tile
from concourse import bacc, mybir
from concourse.bass_utils import run_bass_kernel_spmd
from concourse.masks import make_identity

F32 = mybir.dt.float32
F32R = mybir.dt.float32r
B16 = mybir.dt.bfloat16
AF = mybir.ActivationFunctionType

# Problem constants (hardcoded per contract)
B, C, H, W = 8, 128, 48, 48
CM = 64
HL = WL = 16
L = 256               # reference patches
SM = 10.0
ESC = 1.0e-4
HP = H + 6            # x padded by 3 -> 54
MPH, MPW = 52, 64     # match pad rows 0..49 used (+2 guard), 64-wide rows
APH, APW = 51, 50     # attn pad: rows 0..49 + guard row, 50-wide
NCHUNK = 24           # p' chunks of 128 (2 rows x 64)
JBS = [10, 10, 10, 10, 8]   # j-row blocks for the tconv


def build_program(a1, a2, aa, need_ba, use_prelu=True,
                  mm_dt=F32R, score_dt=F32R):
    """Build the single-core Bass program. Scalars a1/a2/aa baked as imms."""
    nc = bacc.Bacc("TRN2", target_bir_lowering=False, debug=False)

    x = nc.dram_tensor("x", [C, H, W], F32, kind="ExternalInput").ap()
    w1t = nc.dram_tensor("w1t", [C, CM], F32, kind="ExternalInput").ap()
    w2t = nc.dram_tensor("w2t", [C, CM], F32, kind="ExternalInput").ap()
    wat = nc.dram_tensor("wat", [C, C], F32, kind="ExternalInput").ap()
    b1 = nc.dram_tensor("b1", [CM, 1], F32, kind="ExternalInput").ap()
    b2 = nc.dram_tensor("b2", [CM, 1], F32, kind="ExternalInput").ap()
    bar = nc.dram_tensor("bar", [1, C], F32, kind="ExternalInput").ap()
    out = nc.dram_tensor("out", [C, 3 * H, 3 * W], F32, kind="ExternalOutput").ap()

    def prelu_evac(out_ap, in_ap, alpha, bias=0.0):
        nc.scalar.activation(out_ap, in_ap, AF.Prelu,
                             bias=bias, scale=1.0, alpha=float(alpha))

    with tile.TileContext(nc) as tc:
        import contextlib
        ctx = contextlib.ExitStack()
        with ctx:
            consts = ctx.enter_context(tc.tile_pool(name="consts", bufs=1))
            work = ctx.enter_context(tc.tile_pool(name="work", bufs=3))
            small = ctx.enter_context(tc.tile_pool(name="small", bufs=4))
            gpool = ctx.enter_context(tc.tile_pool(name="gpool", bufs=4))
            stpool = ctx.enter_context(tc.tile_pool(name="stage", bufs=2))
            ps_acc = ctx.enter_context(
                tc.tile_pool(name="ps_acc", bufs=3, space="PSUM"))
            ps_aux = ctx.enter_context(
                tc.tile_pool(name="ps_aux", bufs=3, space="PSUM"))

            # ---- constants / inputs in SBUF ----
            w1t_sb = consts.tile([C, CM], F32)
            w2t_sb = consts.tile([C, CM], F32)
            wat_sb = consts.tile([C, C], F32)
            b1_sb = consts.tile([CM, 1], F32)
            b2_sb = consts.tile([CM, 1], F32)
            ones64 = consts.tile([CM, 1], F32)
            ones1 = consts.tile([1, C], F32)
            ident = consts.tile([C, C], F32)
            x_sb = consts.tile([C, H * W], F32)
            xpad = consts.tile([C, HP, HP], F32)
            mpad = consts.tile([CM, MPH, MPW], score_dt)
            rpad = consts.tile([CM, 18, 18], F32)
            xsub = consts.tile([C, L], F32)
            wat_bf = consts.tile([C, C], B16)
            rwin = consts.tile([CM, 9, L], score_dt)
            invb = consts.tile([C, L], F32)
            apad = [consts.tile([C, APH, APW], mm_dt, name=f"apad{i}",
                                tag=f"apad{i}") for i in range(2)]
            if need_ba:
                bar_sb = consts.tile([1, C], F32)
                nc.sync.dma_start(bar_sb[:], bar)

            nc.sync.dma_start(w1t_sb[:], w1t)
            nc.sync.dma_start(w2t_sb[:], w2t)
            nc.sync.dma_start(wat_sb[:], wat)
            nc.sync.dma_start(b1_sb[:], b1)
            nc.sync.dma_start(b2_sb[:], b2)
            nc.gpsimd.memset(ones64[:], 1.0)
            nc.gpsimd.memset(ones1[:], 1.0)
            make_identity(nc, ident[:])

            for j0 in range(0, 48, 10):
                nj = min(10, 48 - j0)
                nc.sync.dma_start(
                    x_sb[:, j0 * 48:(j0 + nj) * 48],
                    x.rearrange("c h w -> c (h w)")[:, j0 * 48:(j0 + nj) * 48])
            nc.vector.tensor_copy(wat_bf[:], wat_sb[:])
            # x -> xpad interior; zero borders (pad 3)
            nc.gpsimd.memset(xpad[:, 0:3, :], 0.0)
            nc.gpsimd.memset(xpad[:, 51:54, :], 0.0)
            nc.gpsimd.memset(xpad[:, 3:51, 0:3], 0.0)
            nc.gpsimd.memset(xpad[:, 3:51, 51:54], 0.0)
            nc.sync.dma_start(xpad[:, 3:51, 3:51], x)

            # match pad zeros: top row, rows>=49, col 0, cols >= 49
            nc.vector.memset(mpad[:, 0, :].bitcast(F32), 0.0)
            nc.vector.memset(mpad[:, 49:52, :].bitcast(F32), 0.0)
            nc.vector.memset(mpad[:, 1:49, 0].bitcast(F32), 0.0)
            nc.vector.memset(mpad[:, 1:49, 49:64].bitcast(F32), 0.0)
            nc.gpsimd.memset(rpad[:], 0.0)
            for i in range(2):
                nc.vector.memset(apad[i][:, 0, :].bitcast(F32), 0.0)
                nc.vector.memset(apad[i][:, 49:51, :].bitcast(F32), 0.0)
                nc.vector.memset(apad[i][:, 1:49, 0].bitcast(F32), 0.0)
                nc.vector.memset(apad[i][:, 1:49, 49].bitcast(F32), 0.0)

            # ---- phase A: small convs ----
            # match = prelu(w1 @ x + b1) -> mpad interior
            for jb, j0 in enumerate(range(0, 48, 10)):
                nj = min(10, 48 - j0)
                mps = ps_aux.tile([CM, 512], F32, tag="aux")
                nc.tensor.matmul(mps[:, :nj * 48], w1t_sb[:],
                                 x_sb[:, j0 * 48:(j0 + nj) * 48],
                                 start=True, stop=True)
                dst = mpad[:, 1 + j0:1 + j0 + nj, 1:49]
                prelu_evac(dst, mps[:, :nj * 48], a1, bias=b1_sb[:])

            # ref = prelu(w2 @ x_sub + b2) -> rpad interior
            nc.vector.tensor_copy(xsub[:], xpad[:, 4:52:3, 4:52:3])
            rps = ps_aux.tile([CM, 512], F32, tag="aux")
            nc.tensor.matmul(rps[:, :L], w2t_sb[:], xsub[:],
                             start=True, stop=True)
            prelu_evac(rpad[:, 1:17, 1:17], rps[:, :L], a2, bias=b2_sb[:])
            # contiguous (dy,dx) windows of rpad for the score rhs (f32r)
            for k, (dy, dx) in enumerate(
                    (a, b) for a in range(3) for b in range(3)):
                nc.vector.tensor_copy(
                    rwin[:, k, :].rearrange("p (a b) -> p a b", a=16),
                    rpad[:, dy:dy + 16, dx:dx + 16])

            # nrm2[l] = sum_{cm,dy,dx} rpad[cm, lh+dy, lw+dx]^2
            sq = work.tile([CM, 18 * 18], F32)
            rpf = rpad[:].rearrange("p a b -> p (a b)")
            nc.vector.tensor_mul(sq[:], rpf, rpf)
            n2ps = ps_aux.tile([1, 512], F32, tag="aux")
            nc.tensor.matmul(n2ps[:, :324], ones64[:], sq[:],
                             start=True, stop=True)
            s2 = small.tile([1, 18, 18], F32)
            nc.vector.tensor_copy(s2[:], n2ps[:, :324].rearrange(
                "p (a b) -> p a b", a=18))
            rs3 = small.tile([1, 18, 16], F32)
            nc.vector.tensor_add(rs3[:], s2[:, :, 0:16], s2[:, :, 1:17])
            nc.vector.tensor_add(rs3[:], rs3[:], s2[:, :, 2:18])
            n2 = small.tile([1, 16, 16], F32)
            nc.vector.tensor_add(n2[:], rs3[:, 0:16, :], rs3[:, 1:17, :])
            nc.vector.tensor_add(n2[:], n2[:], rs3[:, 2:18, :])
            nrm = small.tile([1, L], F32)
            nc.scalar.activation(nrm[:], n2[:].rearrange("p a b -> p (a b)"),
                                 AF.Sqrt, bias=0.0, scale=1.0)
            nc.vector.tensor_scalar_max(out=nrm[:], in0=nrm[:], scalar1=ESC)
            inv = small.tile([1, L], F32)
            nc.vector.reciprocal(inv[:], nrm[:])
            inv10 = small.tile([1, L], F32)
            nc.scalar.mul(inv10[:], inv[:], SM)
            ibps = ps_aux.tile([C, 512], F32, tag="aux")
            nc.tensor.matmul(ibps[:, :L], ones1[:], inv10[:],
                             start=True, stop=True)
            nc.vector.tensor_copy(invb[:], ibps[:, :L])

            # ---- phase B: score + softmax + transpose ----
            mpad_f = mpad.rearrange("p a b -> p (a b)")
            for t in range(NCHUNK):
                sps = ps_acc.tile([C, 512], F32, tag="acc")
                for k, (dy, dx) in enumerate(
                        (a, b) for a in range(3) for b in range(3)):
                    o = (2 * t + dy) * MPW + dx
                    nc.tensor.matmul(sps[:, :L], mpad_f[:, o:o + 128],
                                     rwin[:, k, :],
                                     start=(k == 0), stop=(k == 8))
                scored = work.tile([C, L], F32, tag="scored")
                nc.vector.tensor_mul(scored[:], sps[:, :L], invb[:])
                nmax = small.tile([C, 1], F32, tag="nmax")
                nc.vector.reduce_max(nmax[:], scored[:],
                                     axis=mybir.AxisListType.X, negate=True)
                esb = work.tile([C, L], F32, tag="esb")
                rsum = small.tile([C, 1], F32, tag="rsum")
                nc.scalar.activation(esb[:], scored[:], AF.Exp,
                                     bias=nmax[:], scale=1.0, accum_out=rsum[:])
                rinv = small.tile([C, 1], F32, tag="rinv")
                nc.vector.reciprocal(rinv[:], rsum[:])
                attnT = work.tile([C, L], F32, tag="attnT")
                nc.scalar.activation(attnT[:], esb[:], AF.Copy,
                                     bias=0.0, scale=rinv[:])
                tps = ps_aux.tile([C, 512], F32, tag="aux")
                for lc in range(2):
                    nc.tensor.transpose(tps[:, lc * 128:lc * 128 + 128],
                                        attnT[:, lc * 128:lc * 128 + 128],
                                        ident[:])
                for lc in range(2):
                    src = tps[:, lc * 128:lc * 128 + 128].rearrange(
                        "l (r w) -> l r w", r=2)[:, :, :48]
                    dst = apad[lc][:, 1 + 2 * t:3 + 2 * t, 1:49]
                    nc.scalar.copy(dst, src)

            # ---- phase C: dynamic tconv as 9 residue grids ----
            units = [(m, n, ch) for m in range(3) for n in range(3)
                     for ch in range(2)]
            apf = [apad[i].rearrange("c a b -> c (a b)") for i in range(2)]
            for rh in range(3):
                stage = stpool.tile([C, 48, 48, 3], F32)
                for rw in range(3):
                    # contiguous n-shifted residue-(rh,rw) downsample grids:
                    # drn[n][c, a, lw] = xpad[c, 3a+rh, 3(lw+n)+rw]
                    drn = []
                    for n in range(3):
                        d = gpool.tile([C, 18, 16], B16, name=f"drn{n}",
                                       tag=f"drn{n}")
                        nc.vector.tensor_copy(
                            d[:], xpad[:, rh:rh + 52:3,
                                       rw + 3 * n:rw + 3 * n + 46:3])
                        drn.append(d.rearrange("c a b -> c (a b)"))
                    # G production: 18 units -> 5 quads of [128l, 4*128c]
                    quads = []
                    for q in range(5):
                        gps = ps_acc.tile([C, 512], F32, tag="acc")
                        nslot = min(4, 18 - 4 * q)
                        for s in range(nslot):
                            m, n, ch = units[4 * q + s]
                            a0 = ch * 8 + m
                            lhs_ap = drn[n][:, a0 * 16:a0 * 16 + 128]
                            nc.tensor.matmul(
                                gps[:, s * 128:s * 128 + 128],
                                lhs_ap, wat_bf[:],
                                start=True, stop=(not need_ba))
                            if need_ba:
                                nc.tensor.matmul(
                                    gps[:, s * 128:s * 128 + 128],
                                    ones1[:], bar_sb[:],
                                    start=False, stop=True)
                        gsb = gpool.tile([C, 512], mm_dt, tag="gq", bufs=6)
                        prelu_evac(gsb[:, :nslot * 128], gps[:, :nslot * 128],
                                   aa, bias=0.0)
                        quads.append(gsb)
                    # tconv: S[c, ji] accumulation over 18 units; rhs is a
                    # flat run of the padded attn plane (shift == offset)
                    for jb, j0 in enumerate(range(0, 48, 10)):
                        nj = JBS[jb]
                        vps = ps_acc.tile([C, 512], F32, tag="acc")
                        for u, (m, n, ch) in enumerate(units):
                            base = (j0 + 2 - m) * APW + (2 - n)
                            lhs = quads[u // 4][:, (u % 4) * 128:
                                                (u % 4) * 128 + 128]
                            nc.tensor.matmul(vps[:, :nj * APW], lhs,
                                             apf[ch][:, base:base + nj * APW],
                                             start=(u == 0), stop=(u == 17))
                        src = vps[:, :nj * APW].rearrange(
                            "c (j i) -> c j i", j=nj)[:, :, :48]
                        dst = stage[:, j0:j0 + nj, :, rw]
                        if (jb + rw) % 2 == 0:
                            nc.scalar.activation(dst, src, AF.Copy,
                                                 bias=0.0, scale=1.0 / 6.0)
                        else:
                            nc.vector.tensor_scalar_mul(
                                out=dst, in0=src, scalar1=1.0 / 6.0)
                        if rw == 2:
                            out_r = out.rearrange(
                                "c (j r) q -> c r j q", r=3)[:, rh]
                            nc.sync.dma_start(
                                out_r[:, j0:j0 + nj, :],
                                stage[:, j0:j0 + nj].rearrange(
                                    "c j i r -> c j (i r)"))
    nc.compile()
    return nc


_CACHE = {}


def _get_program(key):
    if key not in _CACHE:
        _CACHE[key] = build_program(*key)
    return _CACHE[key]


def kernel(x, w1, b1, a1, w2, b2, a2, wa, ba, aa):
    x = np.ascontiguousarray(np.asarray(x, dtype=np.float32))
    w1 = np.asarray(w1, dtype=np.float32)
    w2 = np.asarray(w2, dtype=np.float32)
    wa = np.asarray(wa, dtype=np.float32)
    b1 = np.asarray(b1, dtype=np.float32).reshape(CM, 1)
    b2 = np.asarray(b2, dtype=np.float32).reshape(CM, 1)
    ba = np.asarray(ba, dtype=np.float32).reshape(1, C)
    need_ba = bool(np.any(ba != 0.0))
    key = (float(a1), float(a2), float(aa), need_ba)
    nc = _get_program(key)

    common = {
        "w1t": np.ascontiguousarray(w1.T),
        "w2t": np.ascontiguousarray(w2.T),
        "wat": np.ascontiguousarray(wa.T),
        "b1": b1, "b2": b2, "bar": ba,
    }
    in_maps = [dict(common, x=x[b]) for b in range(B)]
    res = run_bass_kernel_spmd(nc, in_maps, core_ids=list(range(B)))
    return np.stack([res.results[b]["out"] for b in range(B)])

